# revision 12
# baseline (speedup 1.0000x reference)
"""Causal self-attention (dense transformer block) on 8 Trainium2 NeuronCores.

Sharding: tensor-parallel over heads. Each core computes qkv + RoPE + causal
attention for 2 of the 16 heads (all 4 batches), then its partial output
projection (contraction over its 256 y-channels). Host sums the 8 partials.

Matmul dtypes: float32r (tf32-class, 1 cyc/row) for qkv/QK/proj; bf16 for
probs*V and the rowsum ones-matmul. Softmax normalization is delayed: P=exp(s)
unnormalized, rowsums collected by a ones-vector matmul, and yT is scaled by
1/rowsum (broadcast across partitions via a K=1 matmul) before projection.
"""

import sys
import numpy as np

sys.path.insert(0, "/opt/trn_rl_repo")

import ml_dtypes  # noqa: E402

import concourse.bacc as bacc  # noqa: E402
import concourse.mybir as mybir  # noqa: E402
from concourse.tile import TileContext  # noqa: E402
from concourse.bass_utils import run_bass_kernel_spmd  # noqa: E402

F32 = mybir.dt.float32
F32R = mybir.dt.float32r
BF16 = mybir.dt.bfloat16

HD = 128          # head dim
D2 = HD // 2      # rope freq count
HPC = 2           # heads per core
ROPE_BASE = 10000.0
N_CORES = 8


def build_nc(B, T, C, debug=False):
    """Build the per-core SPMD program. C = contraction dim (model width)."""
    CS = C // 128         # number of 128-contraction tiles
    TT = T // 128         # t-tiles per batch
    NW = T // 512         # q-windows per batch (window = 512 queries)
    QKF = HPC * 2 * HD    # qk channels per core (512)
    VF = HPC * HD         # v channels per core (256)
    SLAB_T = 256          # tokens per x-slab DMA
    TPS = SLAB_T // 128   # t-tiles per slab

    nc = bacc.Bacc(name="csa_tp")

    x_in = nc.dram_tensor("xTr", [B, CS, 128, T], F32, kind="ExternalInput")
    wa_in = nc.dram_tensor("waT", [CS, 128, QKF + VF], F32, kind="ExternalInput")
    wp_in = nc.dram_tensor("wpT", [HPC, HD, C], F32, kind="ExternalInput")
    cos_in = nc.dram_tensor("cosN", [T, D2], F32, kind="ExternalInput")
    sin_in = nc.dram_tensor("sinN", [T, D2], F32, kind="ExternalInput")
    mask_in = nc.dram_tensor("cmask", [4, 128, 512], BF16, kind="ExternalInput")
    onesc_in = nc.dram_tensor("onesc", [128, 1], BF16, kind="ExternalInput")
    onesr_in = nc.dram_tensor("onesr", [1, 128], F32, kind="ExternalInput")
    id_in = nc.dram_tensor("ident", [128, 128], F32, kind="ExternalInput")
    out = nc.dram_tensor("out", [B, T, C], F32, kind="ExternalOutput")
    if debug:
        dbg_qt = nc.dram_tensor("dbg_qt", [128, T], F32, kind="ExternalOutput")
        dbg_kt = nc.dram_tensor("dbg_kt", [128, T], F32, kind="ExternalOutput")
        dbg_v = nc.dram_tensor("dbg_v", [128, T], F32, kind="ExternalOutput")
        dbg_yt = nc.dram_tensor("dbg_yt", [128, T], F32, kind="ExternalOutput")
        dbg_p = nc.dram_tensor("dbg_p", [128, 512], F32, kind="ExternalOutput")

    inv_sqrt_hd = 1.0 / float(np.sqrt(HD))

    with TileContext(nc) as tc:
        with tc.tile_pool(name="const", bufs=1) as cpool, \
             tc.tile_pool(name="wpool", bufs=1) as wpool, \
             tc.tile_pool(name="big", bufs=1) as bigpool, \
             tc.tile_pool(name="work", bufs=2) as wk, \
             tc.tile_pool(name="ppool", bufs=3) as ppool, \
             tc.tile_pool(name="ps", bufs=1, space="PSUM") as ps:

            # ---- resident constants / weights ----
            cos_sb = cpool.tile([128, TT * D2], F32)
            sin_sb = cpool.tile([128, TT * D2], F32)
            for tt in range(TT):
                nc.sync.dma_start(cos_sb[:, tt * D2:(tt + 1) * D2],
                                  cos_in[tt * 128:(tt + 1) * 128, :])
                nc.sync.dma_start(sin_sb[:, tt * D2:(tt + 1) * D2],
                                  sin_in[tt * 128:(tt + 1) * 128, :])
            mask_sb = cpool.tile([128, 4 * 512], BF16)
            for r in range(4):
                nc.sync.dma_start(mask_sb[:, r * 512:(r + 1) * 512],
                                  mask_in[r, :, :])
            onesc_sb = cpool.tile([128, 1], BF16)
            nc.sync.dma_start(onesc_sb[:], onesc_in[:])
            onesr_sb = cpool.tile([1, 128], F32R)
            nc.sync.dma_start(onesr_sb[:], onesr_in[:].bitcast(F32R))
            id_sb = cpool.tile([128, 128], F32R)
            nc.sync.dma_start(id_sb[:], id_in[:].bitcast(F32R))

            F = QKF + VF
            wa_sb = wpool.tile([128, CS * F], F32R)
            for cs in range(CS):
                nc.sync.dma_start(wa_sb[:, cs * F:(cs + 1) * F],
                                  wa_in[cs, :, :].bitcast(F32R))
            wp_sb = wpool.tile([128, HPC * C], F32R)
            for h in range(HPC):
                nc.sync.dma_start(wp_sb[:, h * C:(h + 1) * C],
                                  wp_in[h, :, :].bitcast(F32R))

            # ---- per-head state (single-buffered across batches) ----
            QT = [bigpool.tile([128, T], F32R, tag=f"qt{h}", name=f"qt{h}")
                  for h in range(HPC)]
            KT = [bigpool.tile([128, T], F32R, tag=f"kt{h}", name=f"kt{h}")
                  for h in range(HPC)]
            V = [bigpool.tile([128, TT * HD], BF16, tag=f"v{h}", name=f"v{h}")
                 for h in range(HPC)]
            YT = [bigpool.tile([128, T], F32R, tag=f"yt{h}", name=f"yt{h}")
                  for h in range(HPC)]

            for b in range(B):
                # ================= Phase A: qkv + rope + transpose ========
                for slab in range(T // SLAB_T):
                    xs = wk.tile([128, CS * SLAB_T], F32R, tag="xslab")
                    t0 = slab * SLAB_T
                    for cs in range(CS):
                        nc.sync.dma_start(
                            xs[:, cs * SLAB_T:(cs + 1) * SLAB_T],
                            x_in[b, cs, :, t0:t0 + SLAB_T].bitcast(F32R))
                    for tts in range(TPS):
                        tt = slab * TPS + tts
                        p_qk = ps.tile([128, QKF], F32, tag="mm")
                        p_v = ps.tile([128, 512], F32, tag="aux")
                        for cs in range(CS):
                            lhs = xs[:, cs * SLAB_T + tts * 128:
                                     cs * SLAB_T + tts * 128 + 128]
                            nc.tensor.matmul(
                                p_qk[:], lhs,
                                wa_sb[:, cs * (QKF + VF):cs * (QKF + VF) + QKF],
                                start=(cs == 0), stop=(cs == CS - 1))
                            nc.tensor.matmul(
                                p_v[:, 0:VF], lhs,
                                wa_sb[:, cs * (QKF + VF) + QKF:
                                      (cs + 1) * (QKF + VF)],
                                start=(cs == 0), stop=(cs == CS - 1))
                        # v evacuation (natural layout, bf16)
                        for h in range(HPC):
                            nc.scalar.copy(
                                V[h][:, tt * HD:(tt + 1) * HD],
                                p_v[:, h * HD:(h + 1) * HD])
                        # rope on the 4 qk channel blocks (evens-first perm)
                        cosb = cos_sb[:, tt * D2:(tt + 1) * D2] \
                            .unsqueeze(1).to_broadcast([128, 4, D2])
                        sinb = sin_sb[:, tt * D2:(tt + 1) * D2] \
                            .unsqueeze(1).to_broadcast([128, 4, D2])
                        qkr = wk.tile([128, QKF], F32R, tag="qkr")
                        qkr_e = qkr[:].rearrange(
                            "p (blk half i) -> p blk half i", blk=4, half=2)[:, :, 0, :]
                        qkr_o = qkr[:].rearrange(
                            "p (blk half i) -> p blk half i", blk=4, half=2)[:, :, 1, :]
                        s_e = p_qk[:].rearrange(
                            "p (blk half i) -> p blk half i", blk=4, half=2)[:, :, 0, :]
                        s_o = p_qk[:].rearrange(
                            "p (blk half i) -> p blk half i", blk=4, half=2)[:, :, 1, :]
                        tmp = wk.tile([128, 4 * D2], F32, tag="rtmp")
                        tmpv = tmp[:].rearrange("p (blk i) -> p blk i", blk=4)
                        nc.vector.tensor_mul(qkr_e, s_e, cosb)
                        nc.vector.tensor_mul(qkr_o, s_e, sinb)
                        nc.vector.tensor_mul(tmpv, s_o, sinb)
                        nc.vector.tensor_sub(qkr_e, qkr_e, tmpv)
                        tmp2 = wk.tile([128, 4 * D2], F32, tag="rtmp2")
                        tmp2v = tmp2[:].rearrange("p (blk i) -> p blk i", blk=4)
                        nc.vector.tensor_mul(tmp2v, s_o, cosb)
                        nc.vector.tensor_add(qkr_o, qkr_o, tmp2v)
                        # transpose 4 blocks -> QT/KT
                        dsts = [QT[0], QT[1], KT[0], KT[1]]
                        p_t = ps.tile([128, 512], F32, tag="acc")
                        for j in range(4):
                            nc.tensor.transpose(
                                p_t[:, j * 128:(j + 1) * 128].bitcast(F32R),
                                qkr[:, j * 128:(j + 1) * 128], id_sb[:])
                        for j in range(4):
                            nc.scalar.copy(
                                dsts[j][:, tt * 128:(tt + 1) * 128],
                                p_t[:, j * 128:(j + 1) * 128].bitcast(F32R))

                if debug and b == 0:
                    for nm, dst, src, cast in (
                            ("q", dbg_qt, QT[0], True),
                            ("k", dbg_kt, KT[0], True),
                            ("v", dbg_v, V[0], False)):
                        for wdb in range(T // 512):
                            stg = wk.tile([128, 512], F32, tag="dbgstg",
                                          name=f"dstg{nm}{wdb}")
                            s = src[:, wdb * 512:(wdb + 1) * 512]
                            nc.vector.tensor_copy(
                                stg[:], s.bitcast(F32) if cast else s)
                            nc.sync.dma_start(
                                dst[:, wdb * 512:(wdb + 1) * 512], stg[:])

                # ================= Phase B: causal attention ==============
                for h in range(HPC):
                    for w in range(NW):
                        nkb = 4 * w + 4   # k-blocks for this window
                        p_y = ps.tile([128, 512], F32, tag="acc")
                        p_rs = ps.tile([1, 512], F32, tag="rs")
                        for kb in range(nkb):
                            p_s = ps.tile([128, 512], F32, tag="mm")
                            nc.tensor.matmul(
                                p_s[:], KT[h][:, kb * 128:(kb + 1) * 128],
                                QT[h][:, w * 512:(w + 1) * 512],
                                start=True, stop=True)
                            P = ppool.tile([128, 512], BF16, tag="P")
                            nc.scalar.activation(
                                P[:], p_s[:], mybir.ActivationFunctionType.Exp,
                                scale=inv_sqrt_hd)
                            rel = kb - 4 * w
                            if rel >= 0:
                                nc.vector.tensor_mul(
                                    P[:], P[:],
                                    mask_sb[:, (rel) * 512:(rel + 1) * 512])
                            if debug and b == 0 and h == 0 and w == 0 and kb == 0:
                                pstg = wk.tile([128, 512], F32, tag="dbgstg",
                                               name="pstg")
                                nc.vector.tensor_copy(pstg[:], P[:])
                                nc.sync.dma_start(dbg_p[:], pstg[:])
                            nc.tensor.matmul(
                                p_rs[:], onesc_sb[:], P[:],
                                start=(kb == 0), stop=(kb == nkb - 1))
                            nc.tensor.matmul(
                                p_y[:], V[h][:, kb * HD:(kb + 1) * HD], P[:],
                                start=(kb == 0), stop=(kb == nkb - 1))
                        # normalize: yT *= 1/rowsum (broadcast via K=1 matmul)
                        rec = wk.tile([1, 512], F32, tag="rec")
                        nc.vector.reciprocal(rec[:], p_rs[:])
                        rec_r = wk.tile([1, 512], F32R, tag="recr")
                        nc.vector.tensor_copy(rec_r[:], rec[:])
                        p_rb = ps.tile([128, 512], F32, tag="aux")
                        nc.tensor.matmul(p_rb[:], onesr_sb[:], rec_r[:],
                                         start=True, stop=True)
                        rec_sb = wk.tile([128, 512], F32, tag="recsb")
                        nc.scalar.copy(rec_sb[:], p_rb[:])
                        nc.vector.tensor_mul(
                            YT[h][:, w * 512:(w + 1) * 512], p_y[:], rec_sb[:])

                if debug and b == 0:
                    for wdb in range(T // 512):
                        ystg = wk.tile([128, 512], F32, tag="dbgstg",
                                       name=f"ystg{wdb}")
                        nc.vector.tensor_copy(
                            ystg[:],
                            YT[0][:, wdb * 512:(wdb + 1) * 512].bitcast(F32))
                        nc.sync.dma_start(
                            dbg_yt[:, wdb * 512:(wdb + 1) * 512], ystg[:])

                # ================= Phase C: output projection =============
                OCW = min(512, C)
                for tt in range(TT):
                    for oc in range(C // OCW):
                        p_o = ps.tile([128, 512], F32, tag="mm")
                        for h in range(HPC):
                            nc.tensor.matmul(
                                p_o[:, 0:OCW],
                                YT[h][:, tt * 128:(tt + 1) * 128],
                                wp_sb[:, h * C + oc * OCW:h * C + (oc + 1) * OCW],
                                start=(h == 0), stop=(h == HPC - 1))
                        og = wk.tile([128, 512], F32, tag="ostg")
                        nc.scalar.copy(og[:, 0:OCW], p_o[:, 0:OCW])
                        nc.sync.dma_start(
                            out[b, tt * 128:(tt + 1) * 128,
                                oc * OCW:(oc + 1) * OCW], og[:, 0:OCW])

    nc.finalize()
    return nc


def host_prep(x, w_attn, w_proj, n_cores=N_CORES):
    """Prepare per-core input maps."""
    B, T, C = x.shape
    H = C // HD
    hpc = H // n_cores
    assert hpc == HPC
    d = D2

    perm = np.concatenate([np.arange(0, HD, 2), np.arange(1, HD, 2)])
    xTr = np.ascontiguousarray(x.transpose(0, 2, 1)).reshape(B, C // 128, 128, T)

    theta = 1.0 / (ROPE_BASE ** (2.0 * np.arange(d, dtype=np.float64) / HD))
    t = np.arange(T, dtype=np.float64)
    freqs = np.outer(t, theta)
    cosN = np.cos(freqs).astype(np.float32)
    sinN = np.sin(freqs).astype(np.float32)

    cmask = np.zeros((4, 128, 512), dtype=ml_dtypes.bfloat16)
    dk = np.arange(128)[:, None]
    dq = np.arange(512)[None, :]
    for rel in range(4):
        cmask[rel] = (128 * rel + dk <= dq).astype(ml_dtypes.bfloat16)

    onesc = np.ones((128, 1), dtype=ml_dtypes.bfloat16)
    onesr = np.ones((1, 128), dtype=np.float32)
    ident = np.eye(128, dtype=np.float32)

    in_maps = []
    for m in range(n_cores):
        rows = []
        for part in range(3):  # q, k, v blocks of w_attn
            for hh in range(HPC):
                blk = w_attn[part * C + (m * HPC + hh) * HD:
                             part * C + (m * HPC + hh) * HD + HD]
                if part < 2:
                    blk = blk[perm]
                rows.append(blk)
        wsel = np.concatenate(rows, axis=0)          # [768, C]
        waT = np.ascontiguousarray(wsel.T).reshape(C // 128, 128, wsel.shape[0])
        wpT = np.empty((HPC, HD, C), dtype=np.float32)
        for hh in range(HPC):
            c0 = (m * HPC + hh) * HD
            wpT[hh] = np.ascontiguousarray(w_proj[:, c0:c0 + HD].T)
        in_maps.append({
            "xTr": xTr, "waT": waT, "wpT": wpT,
            "cosN": cosN, "sinN": sinN, "cmask": cmask,
            "onesc": onesc, "onesr": onesr, "ident": ident,
        })
    return in_maps


_NC_CACHE = {}


def kernel(x, w_attn, w_proj):
    x = np.asarray(x, dtype=np.float32)
    w_attn = np.asarray(w_attn, dtype=np.float32)
    w_proj = np.asarray(w_proj, dtype=np.float32)
    B, T, C = x.shape

    key = (B, T, C)
    if key not in _NC_CACHE:
        _NC_CACHE[key] = build_nc(B, T, C)
    nc = _NC_CACHE[key]

    in_maps = host_prep(x, w_attn, w_proj)
    res = run_bass_kernel_spmd(nc, in_maps, core_ids=list(range(N_CORES)))
    acc = res.results[0]["out"].astype(np.float32)
    for r in res.results[1:]:
        acc += r["out"]
    return acc
